# revision 1
# baseline (speedup 1.0000x reference)
"""Dynamic spiral pool (gnn_message_passing) TRN2 kernel — 8-core SPMD.

Self-contained: hardcodes shapes from the problem spec
  x [4, 50000, 64] f32, indices [50000, 16] i64, ro_w [1, 64], ro_b [1],
  gamma/beta [64] -> out [4, 50000, 64] f32.

Math (per batch b, node n):
  g[j] = x[b, idx[n,j], :]
  s    = min(|mean_j(g) . ro_w + ro_b| * 16, 15)
  w[j] = clamp(s - j + 1, 0, 1)        # continuous form of the ref's
  y    = sum_j w[j] * g[j]             # cumsum + linear interp
  out  = GroupNorm(4 groups over (n, c_in_group))(y) * gamma + beta

Distribution: nodes sharded 8 ways (6250/core); x replicated as a
node-major record table rec[N+1, 320] (x for all 4 batches + d slot) so
one 1280 B dma_gather descriptor fetches everything for a neighbor.
Stages per core:
  0) d[b,i] = x[b,i].ro_w via PE transpose+matvec; AllGather d;
     write d into the record table tails.
  1) per 128-node block: two 1024-index dma_gathers (int16 indices
     biased by N/2; host permutes node->slot so each instruction ends
     on a non-negative index) into a (slot s, j) partition layout;
     s from gathered d via a block-ones PE matmul; pool weights applied
     as one DVE tensor_tensor; j-reduction via one-hot block PE matmuls
     accumulating into PSUM so y lands node-linear on 128 partitions.
  2) GroupNorm stats per block, AllReduce (128 B), scale/bias apply,
     node-major output write; host un-permutes and reassembles.
"""

import sys

if "/opt/trn_rl_repo" not in sys.path:
    sys.path.insert(0, "/opt/trn_rl_repo")

import numpy as np
import concourse.bass as bass
import concourse.bacc as bacc
import concourse.tile as tile
from concourse import mybir
from concourse.bass_utils import run_bass_kernel_spmd

F32 = mybir.dt.float32
I16 = mybir.dt.int16
AF = mybir.ActivationFunctionType
ALU = mybir.AluOpType
AXL = mybir.AxisListType

B, C, K, G = 4, 64, 16, 4
CG = C // G
REC = B * C            # 256
RECF = 320             # record row f32 (x 256 | d 4 | pad 60) = 1280 B
NSLOT = 16
LIDX = 128             # idx cols per block (2 halves x 64)
NCORES = 8
N = 50000
NB = 49                # 128-node blocks per core
NS = N // NCORES       # 6250
NSP = NB * 128         # 6272
BIAS = N // 2
CNT = float(N * CG)


def _mk_ap(base, dims):
    return bass.AP(tensor=base.tensor, offset=base.offset,
                   ap=[base.ap[0]] + dims)


def _build():
    nc = bacc.Bacc(None, target_bir_lowering=False, debug=False)

    rec = nc.declare_dram_parameter("rec", [N + 1, RECF], F32, isOutput=False)
    offs_d = nc.declare_dram_parameter("offs", [128, NB * LIDX], I16,
                                       isOutput=False)
    xs = nc.declare_dram_parameter("xs", [NSP, REC], F32, isOutput=False)
    id128 = nc.declare_dram_parameter("id128", [128, 128], F32, isOutput=False)
    ones16 = nc.declare_dram_parameter("ones16", [128, 128], F32,
                                       isOutput=False)
    mm64 = nc.declare_dram_parameter("mm64", [128, 512], F32, isOutput=False)
    onescol = nc.declare_dram_parameter("onescol", [128, 1], F32,
                                        isOutput=False)
    w01 = nc.declare_dram_parameter("w01", [128, 4], F32, isOutput=False)
    w23 = nc.declare_dram_parameter("w23", [128, 4], F32, isOutput=False)
    jm1 = nc.declare_dram_parameter("jm1", [128, 1], F32, isOutput=False)
    bb0 = nc.declare_dram_parameter("bb0", [128, 1], F32, isOutput=False)
    gamma = nc.declare_dram_parameter("gamma", [1, C], F32, isOutput=False)
    beta = nc.declare_dram_parameter("beta", [1, C], F32, isOutput=False)
    yt = nc.declare_dram_parameter("yt", [NSP, REC], F32, isOutput=True)

    with tile.TileContext(nc) as tc:
        with (
            tc.tile_pool(name="consts", bufs=1) as consts,
            tc.tile_pool(name="dram", bufs=1, space="DRAM") as dram,
        ):
            id128s = consts.tile([128, 128], F32)
            ones16s = consts.tile([128, 128], F32)
            mm64s = consts.tile([128, 512], F32)
            onescols = consts.tile([128, 1], F32)
            w01s = consts.tile([128, 4], F32)
            w23s = consts.tile([128, 4], F32)
            jm1s = consts.tile([128, 1], F32)
            bb0s = consts.tile([128, 1], F32)
            gammas = consts.tile([128, C], F32)
            betas = consts.tile([128, C], F32)
            offs_t = consts.tile([128, NB * LIDX], I16)
            dloc = consts.tile([128, NB * 4], F32)
            yall = consts.tile([128, NB * REC], F32)
            SS = consts.tile([128, 32], F32)
            epst = consts.tile([128, 1], F32)
            zrow = consts.tile([1, 4], F32)

            for dst, src in [
                (id128s, id128), (ones16s, ones16), (mm64s, mm64),
                (onescols, onescol), (w01s, w01), (w23s, w23),
                (jm1s, jm1), (bb0s, bb0), (offs_t, offs_d),
            ]:
                nc.sync.dma_start(out=dst[:], in_=src[:])
            nc.gpsimd.dma_start(
                out=gammas[:], in_=bass.AP(
                    tensor=gamma[:].tensor, offset=gamma[:].offset,
                    ap=[[0, 128], [1, C]]))
            nc.gpsimd.dma_start(
                out=betas[:], in_=bass.AP(
                    tensor=beta[:].tensor, offset=beta[:].offset,
                    ap=[[0, 128], [1, C]]))
            nc.vector.memset(SS[:], 0.0)
            nc.vector.memset(zrow[:], 0.0)
            nc.vector.memset(epst[:], 1e-5)

            dslice = dram.tile([NSP, 4], F32)
            dall = dram.tile([N + 1, 4], F32)
            stat_in = dram.tile([1, 32], F32)
            stat_out = dram.tile([1, 32], F32)

            # ---------------- stage 0: d = x . ro_w ----------------
            with (
                tc.tile_pool(name="s0", bufs=2) as s0p,
                tc.tile_pool(name="s0ps", bufs=1, space="PSUM") as s0ps,
            ):
                for blk in range(NB):
                    X = s0p.tile([128, REC], F32)
                    nc.sync.dma_start(
                        out=X[:], in_=xs[blk * 128:(blk + 1) * 128, :])
                    dT = s0ps.tile([128, 4], F32, tag="dT")
                    ds = s0p.tile([4, 128], F32, tag="ds")
                    pd = s0ps.tile([4, 128], F32, tag="pd")
                    Ts_halves = []
                    for half in range(2):
                        Tp = s0ps.tile([128, 128], F32, tag="Tp")
                        nc.tensor.transpose(
                            out=Tp[:], in_=X[:, half * 128:(half + 1) * 128],
                            identity=id128s[:])
                        Ts = s0p.tile([128, 128], F32, tag="Ts")
                        nc.scalar.copy(out=Ts[:], in_=Tp[:])
                        Ts_halves.append(Ts)
                    nc.tensor.matmul(out=pd[:, :], lhsT=w01s[:],
                                     rhs=Ts_halves[0][:], start=True,
                                     stop=False)
                    nc.tensor.matmul(out=pd[:, :], lhsT=w23s[:],
                                     rhs=Ts_halves[1][:], start=False,
                                     stop=True)
                    nc.scalar.copy(out=ds[:], in_=pd[:])
                    nc.tensor.transpose(
                        out=dT[:, :], in_=ds[:, :], identity=id128s[:4, :4])
                    nc.scalar.copy(
                        out=dloc[:, blk * 4:(blk + 1) * 4], in_=dT[:])

            nc.sync.dma_start(
                out=dslice[:].rearrange("(t p) d -> p t d", p=128),
                in_=dloc[:].rearrange("p (t d) -> p t d", d=4))
            nc.sync.dma_start(out=dall[N:N + 1, :], in_=zrow[:])
            nc.gpsimd.collective_compute(
                "AllGather", ALU.bypass,
                replica_groups=[list(range(NCORES))],
                ins=[dslice[:NS, :].opt()],
                outs=[dall[:N, :].opt()],
            )
            nc.sync.dma_start(out=rec[:N + 1, 256:260], in_=dall[:, :])

            # ---------------- stage 1: gather + pool ----------------
            with (
                tc.tile_pool(name="s1", bufs=3) as s1p,
                tc.tile_pool(name="s1ps", bufs=2, space="PSUM") as s1ps,
                tc.tile_pool(name="s1y", bufs=2, space="PSUM") as s1yp,
            ):
                for blk in range(NB):
                    R = s1p.tile([128, NSLOT * RECF], F32, tag="R")
                    for h in range(2):
                        nc.gpsimd.dma_gather(
                            out_ap=R[:, h * 8 * RECF:(h + 1) * 8 * RECF]
                            .rearrange("p (u e) -> p u e", e=RECF),
                            in_ap=rec[BIAS:, :],
                            idxs_ap=offs_t[:, blk * LIDX + h * 64:
                                           blk * LIDX + (h + 1) * 64],
                            num_idxs=1024,
                            num_idxs_reg=1024,
                            elem_size=RECF,
                        )
                    ps = s1ps.tile([128, 64], F32, tag="ps")
                    nc.tensor.matmul(
                        out=ps[:], lhsT=ones16s[:],
                        rhs=_mk_ap(R[:, 256:260], [[RECF, K], [1, 4]]),
                        start=True, stop=True)
                    sabs = s1p.tile([128, 64], F32, tag="sabs")
                    nc.scalar.activation(
                        out=sabs[:], in_=ps[:], func=AF.Abs,
                        bias=bb0s[:], scale=1.0)
                    wt = s1p.tile([128, 64], F32, tag="wt")
                    nc.vector.tensor_scalar(
                        out=wt[:], in0=sabs[:], scalar1=float(K - 1),
                        scalar2=jm1s[:], op0=ALU.min, op1=ALU.subtract)
                    nc.vector.tensor_scalar(
                        out=wt[:], in0=wt[:], scalar1=0.0, scalar2=1.0,
                        op0=ALU.max, op1=ALU.min)
                    g2 = s1p.tile([128, K * REC], F32, tag="g2")
                    r4 = _mk_ap(R[:], [[RECF, K], [C, B], [1, C]])
                    wtb = _mk_ap(wt[:], [[4, K], [1, B], [0, C]])
                    nc.vector.tensor_tensor(
                        out=g2[:].rearrange("p (q b c) -> p q b c", q=K, b=B),
                        in0=r4, in1=wtb, op=ALU.mult)
                    py = s1yp.tile([128, REC], F32, tag="py")
                    for H in range(2):
                        for qq in range(8):
                            nc.tensor.matmul(
                                out=py[H * 64:(H + 1) * 64, :],
                                lhsT=mm64s[:, qq * 64:(qq + 1) * 64],
                                rhs=g2[:, (H * 8 + qq) * REC:
                                        (H * 8 + qq + 1) * REC],
                                start=(qq == 0), stop=(qq == 7))
                    yb = yall[:, blk * REC:(blk + 1) * REC]
                    nc.scalar.copy(out=yb, in_=py[:])
                    sq = s1p.tile([128, REC], F32, tag="sq")
                    nc.vector.tensor_tensor(
                        out=sq[:], in0=yb, in1=yb, op=ALU.mult)
                    s1b = s1p.tile([128, 16], F32, tag="s1b")
                    nc.vector.tensor_reduce(
                        out=s1b[:],
                        in_=yb.rearrange("p (bg cg) -> p bg cg", cg=CG),
                        axis=AXL.X, op=ALU.add)
                    s2b = s1p.tile([128, 16], F32, tag="s2b")
                    nc.vector.tensor_reduce(
                        out=s2b[:],
                        in_=sq[:].rearrange("p (bg cg) -> p bg cg", cg=CG),
                        axis=AXL.X, op=ALU.add)
                    nc.vector.tensor_tensor(
                        out=SS[:, 0:16], in0=SS[:, 0:16], in1=s1b[:],
                        op=ALU.add)
                    nc.vector.tensor_tensor(
                        out=SS[:, 16:32], in0=SS[:, 16:32], in1=s2b[:],
                        op=ALU.add)

                # ---------------- stage 2: groupnorm ----------------
                pst = s1ps.tile([1, 32], F32, tag="pst")
                nc.tensor.matmul(out=pst[:], lhsT=onescols[:], rhs=SS[:],
                                 start=True, stop=True)
                sti = s1p.tile([1, 32], F32, tag="sti")
                nc.scalar.copy(out=sti[:], in_=pst[:])
                nc.sync.dma_start(out=stat_in[:], in_=sti[:])
                nc.gpsimd.collective_compute(
                    "AllReduce", ALU.add,
                    replica_groups=[list(range(NCORES))],
                    ins=[stat_in[:].opt()],
                    outs=[stat_out[:].opt()],
                )
                st = s1p.tile([128, 32], F32, tag="st")
                nc.gpsimd.dma_start(
                    out=st[:], in_=bass.AP(
                        tensor=stat_out[:].tensor, offset=stat_out[:].offset,
                        ap=[[0, 128], [1, 32]]))

                mean = s1p.tile([128, 16], F32, tag="mean")
                nc.scalar.mul(mean[:], st[:, 0:16], 1.0 / CNT)
                var = s1p.tile([128, 16], F32, tag="var")
                nc.scalar.mul(var[:], st[:, 16:32], 1.0 / CNT)
                msq = s1p.tile([128, 16], F32, tag="msq")
                nc.vector.tensor_tensor(
                    out=msq[:], in0=mean[:], in1=mean[:], op=ALU.mult)
                nc.vector.tensor_tensor(
                    out=var[:], in0=var[:], in1=msq[:], op=ALU.subtract)
                rst = s1p.tile([128, 16], F32, tag="rst")
                nc.scalar.activation(out=rst[:], in_=var[:], func=AF.Sqrt,
                                     bias=epst[:], scale=1.0)
                nc.vector.reciprocal(out=rst[:], in_=rst[:])
                A0 = s1p.tile([128, REC], F32, tag="A0")
                nc.vector.tensor_tensor(
                    out=A0[:],
                    in0=_mk_ap(rst[:], [[1, 16], [0, CG]]),
                    in1=_mk_ap(gammas[:], [[0, B], [1, C]]),
                    op=ALU.mult)
                B0 = s1p.tile([128, REC], F32, tag="B0")
                nc.vector.tensor_tensor(
                    out=B0[:],
                    in0=_mk_ap(mean[:], [[1, 16], [0, CG]]),
                    in1=A0[:], op=ALU.mult)
                nc.vector.tensor_tensor(
                    out=B0[:],
                    in0=_mk_ap(betas[:], [[0, B], [1, C]]),
                    in1=B0[:], op=ALU.subtract)

                ya = yall[:].rearrange("p (blk c) -> p blk c", c=REC)
                nc.vector.tensor_tensor(
                    out=ya, in0=ya,
                    in1=_mk_ap(A0[:], [[0, NB], [1, REC]]), op=ALU.mult)
                nc.vector.tensor_tensor(
                    out=ya, in0=ya,
                    in1=_mk_ap(B0[:], [[0, NB], [1, REC]]), op=ALU.add)
                nc.sync.dma_start(
                    out=yt.rearrange("(blk p) c -> p blk c", p=128),
                    in_=ya)

    nc.compile()
    return nc


def _host_prep(x, indices, ro_w, ro_b, gamma, beta):
    rec = np.zeros((N + 1, RECF), dtype=np.float32)
    rec[:N, :REC] = np.ascontiguousarray(x.transpose(1, 0, 2)).reshape(N, REC)
    idx32 = np.asarray(indices, dtype=np.int32)

    j_of_p = np.arange(128) % 16
    id128 = np.eye(128, dtype=np.float32)
    ones16 = np.repeat(np.repeat(np.eye(8, dtype=np.float32), 16, axis=0),
                       16, axis=1)
    mm64 = np.zeros((128, 512), dtype=np.float32)
    kk = np.arange(128)
    for qq in range(8):
        mm64[kk, qq * 64 + qq * 8 + kk // 16] = 1.0
    onescol = np.ones((128, 1), dtype=np.float32)
    wv = np.asarray(ro_w, dtype=np.float32).reshape(C)
    w01 = np.zeros((128, 4), dtype=np.float32)
    w01[:64, 0] = wv
    w01[64:, 1] = wv
    w23 = np.zeros((128, 4), dtype=np.float32)
    w23[:64, 2] = wv
    w23[64:, 3] = wv
    jm1 = (j_of_p.astype(np.float32) - 1.0).reshape(128, 1)
    bb0 = np.full((128, 1),
                  float(K) * float(np.asarray(ro_b).reshape(-1)[0]),
                  dtype=np.float32)
    gam = np.asarray(gamma, dtype=np.float32).reshape(1, C)
    bet = np.asarray(beta, dtype=np.float32).reshape(1, C)

    in_maps = []
    perms = []
    for r in range(NCORES):
        nodes = r * NS + np.arange(NSP)
        valid = nodes < (r + 1) * NS
        nidx = np.where(valid[:, None],
                        idx32[np.minimum(nodes, N - 1)], N)   # [NSP, K]
        # permute node->slot so each 1024-desc gather ends >= 0 (biased)
        cand = nidx[:, K - 1] >= BIAS
        last_slots = (np.arange(NB * 2) // 2) * 128 + \
            ((np.arange(NB * 2) % 2) * 8 + 7) * 8 + 7
        perm = np.empty(NSP, dtype=np.int64)
        cpos = np.nonzero(cand)[0]
        assert len(cpos) >= NB * 2, "not enough tail candidates"
        perm[last_slots] = cpos[:NB * 2]
        rest = np.setdiff1d(np.arange(NSP), cpos[:NB * 2])
        oslots = np.setdiff1d(np.arange(NSP), last_slots, assume_unique=True)
        perm[oslots] = rest
        perms.append(perm)
        pidx = nidx[perm]
        pi = pidx.reshape(NB, 2, 8, 8, K)            # [blk, h, q', s, j]
        L = (pi.reshape(NB, 2, 1024) - BIAS).astype(np.int32)
        W = L.reshape(NB, 2, 64, 16).transpose(0, 1, 3, 2)
        offs = np.tile(W, (1, 1, 8, 1)).transpose(2, 0, 1, 3).reshape(
            128, NB * LIDX).astype(np.int16)
        safe = np.where(valid, np.minimum(nodes, N - 1), N)
        xs = np.ascontiguousarray(rec[safe, :REC])
        in_maps.append({
            "rec": rec, "offs": offs, "xs": xs, "id128": id128,
            "ones16": ones16, "mm64": mm64, "onescol": onescol,
            "w01": w01, "w23": w23, "jm1": jm1, "bb0": bb0,
            "gamma": gam, "beta": bet,
        })
    return in_maps, perms


_NC_CACHE = None


def _get_nc():
    global _NC_CACHE
    if _NC_CACHE is None:
        _NC_CACHE = _build()
    return _NC_CACHE


def run_on_device(inputs, trace=False, trace_cores=None):
    """Run and return (output, BassKernelResults)."""
    x = np.asarray(inputs["x"], dtype=np.float32)
    indices = np.asarray(inputs["indices"])
    ro_w = np.asarray(inputs["ro_w"], dtype=np.float32)
    ro_b = np.asarray(inputs["ro_b"], dtype=np.float32)
    gamma = np.asarray(inputs["gamma"], dtype=np.float32)
    beta = np.asarray(inputs["beta"], dtype=np.float32)
    nc = _get_nc()
    in_maps, perms = _host_prep(x, indices, ro_w, ro_b, gamma, beta)
    res = run_bass_kernel_spmd(nc, in_maps, list(range(NCORES)),
                               trace=trace, trace_cores=trace_cores)
    out = np.empty((B, N, C), dtype=np.float32)
    for r in range(NCORES):
        ytc = res.results[r]["yt"]
        nodes = r * NS + np.arange(NSP)
        pnodes = nodes[perms[r]]
        ok = pnodes < (r + 1) * NS
        out[:, pnodes[ok], :] = ytc[ok].reshape(-1, B, C).transpose(1, 0, 2)
    return out, res


def kernel(**inputs) -> np.ndarray:
    out, _ = run_on_device(inputs, trace=False)
    return out

